# revision 34
# baseline (speedup 1.0000x reference)
"""Trainium2 Bass kernel for masked graph-convolution interaction.

Math (reference):
    wf = node_features @ weight                              # [N, D]
    T[i,d,j] = wf[i,d] * wf[j,d] * mh[i,j]
    S[a,d,j] = sum_i adj[a,i] * T[i,d,j]
    out[a,d] = sum_j S[a,d,j] * mf[a,j] / ncnt[a]^2

Centered-fp8 formulation. With c[j] = mean_i mh[i,j], r[a] = mean_i adj[a,i],
R = mh - c, A' = adj - r:

    out*nc^2 = BULK + F.*H + r.*K            where
    BULK[a,d] = sum_j ( sum_i A'[a,i]wf[i,d] * R[i,j] ) * wf[j,d] * mf[a,j]
    F = adj@wf ; H = (mf.*c)@wf ; G = wf^T@R ; K = mf@(G.*wf^T)^T

The O(N^3 D) BULK runs on-device as fp8 DoubleRow matmuls (2x bf16 rate);
centering halves both operand magnitudes, quartering fp8 quantization error
(~6e-3 rel vs 2.8e-2 naive fp8). Everything O(N^2 D) — wf, the per-row
stationary X8[a] = (adj[a,:]-r)*wf, the post-multiplier SA[a] = wf^T.*mf[a,:],
and the exact correction corr = F.*H + r.*K — is host-prepped (the sharding
hint replicates host-computed wf), so the device pipeline per output row is:

    PE : psum[d,j] = sum_i X8[a][i,d] * R8[i,j]   (8 DoubleRow matmuls)
    DVE: z2 = psum .* SA[a]                       (one [128,1024] multiply)
    ACT: outcol[d] = sum_j z2[d,j]                (free-axis accumulator)

Final: PE transpose of outcols, add corr, scale by 1/nc^2 (host-sent).

Sharding: row-split of a across 8 cores (128 rows each); R8 replicated.
"""

import numpy as np

N = 1024
DIN = 256
DOUT = 128
NCORES = 8
ROWS = N // NCORES  # 128 output rows per core
P = 128

_DTYPE = "fp8-centered"  # informational; test.py prints it

_CACHE = {}


def _build():
    """Build + compile the Bass module (shared across all 8 cores, SPMD)."""
    import concourse.bass as bass
    import concourse.tile as tile
    from concourse import bacc, mybir
    from concourse._compat import axon_active
    from concourse.masks import make_identity

    f32 = mybir.dt.float32
    bf16 = mybir.dt.bfloat16
    f8 = mybir.dt.float8e4
    Copy = mybir.ActivationFunctionType.Copy
    DR = mybir.MatmulPerfMode.DoubleRow

    nc = bacc.Bacc(
        "TRN2",
        target_bir_lowering=False,
        debug=not axon_active(),
        num_devices=NCORES,
    )

    IC = N // P  # 8 i-chunks of 128

    R8_d = nc.dram_tensor("R8", [N, N], f8, kind="ExternalInput").ap()
    # X8 host layout: [a, p, c, d] with i = c*128+p, so each partition's
    # 1KB (IC*DOUT fp8) is one contiguous DMA run
    X8_d = nc.dram_tensor("X8", [ROWS, P, IC * DOUT], f8, kind="ExternalInput").ap()
    SA_d = nc.dram_tensor("SA", [ROWS, DOUT, N], bf16, kind="ExternalInput").ap()
    # corr/inv half-split: [p, h*D+d] = corr[h*64+p, d], so each 64-row half
    # sits on partitions 0-63 (transpose psum outputs must start at part 0)
    corr_d = nc.dram_tensor(
        "corr", [ROWS // 2, 2 * DOUT], f32, kind="ExternalInput"
    ).ap()
    inv2_d = nc.dram_tensor("inv2", [ROWS // 2, 2], f32, kind="ExternalInput").ap()
    out_d = nc.dram_tensor("out", [ROWS, DOUT], f32, kind="ExternalOutput").ap()

    with tile.TileContext(nc) as tc:
        with (
            tc.tile_pool(name="const", bufs=1) as cpool,
            tc.tile_pool(name="x", bufs=6) as xpool,
            tc.tile_pool(name="sa", bufs=6) as sapool,
            tc.tile_pool(name="z", bufs=4) as zpool,
            tc.tile_pool(name="ps", bufs=2, space="PSUM") as spool,
            tc.tile_pool(name="py", bufs=2, space="PSUM") as ypool,
        ):
            # ---- resident tiles + input DMA ----
            # R8 as 4 tiles of 2 i-chunks each: the first DoubleRow matmul
            # only waits on tile 0 (256KB), not the whole 1MB
            R8_sbs = [
                cpool.tile([P, 2, N], f8, tag=f"R8_{c4}", name=f"R8_{c4}")
                for c4 in range(IC // 2)
            ]
            for c4 in range(IC // 2):
                for s in range(2):
                    c = 2 * c4 + s
                    if c4 == 0:
                        # first tile gates the pipeline start: spray each
                        # chunk across 4 queues via partition-quarter DMAs
                        for q in range(4):
                            ps = slice(q * 32, (q + 1) * 32)
                            nc.sync.dma_start(
                                R8_sbs[c4][ps, s, :],
                                R8_d[c * P + q * 32 : c * P + (q + 1) * 32, :],
                            )
                    else:
                        nc.sync.dma_start(
                            R8_sbs[c4][:, s, :], R8_d[c * P : (c + 1) * P, :]
                        )
            corr_sb = cpool.tile([ROWS // 2, 2 * DOUT], f32, tag="corr")
            inv_sb = cpool.tile([ROWS // 2, 2], f32, tag="inv")
            nc.sync.dma_start(corr_sb[:], corr_d[:])
            nc.sync.dma_start(inv_sb[:], inv2_d[:])
            id_sb = cpool.tile([P, P], f32, tag="ident")

            # ---- main loop over the 128 output rows ----
            make_identity(nc, id_sb[:])
            outcols_sb = cpool.tile([P, ROWS], f32, tag="outcols")
            out_sb = cpool.tile([ROWS // 2, 2 * DOUT], f32, tag="out_sb")
            for a in range(ROWS):
                # X8[a] as [p, (c, d)] — contiguous 1KB per partition;
                # first rows split across 2 queues to cut pipeline fill
                x_t = xpool.tile([P, IC, DOUT], f8, tag="X")
                nsplit = 2 if a < 2 else 1
                for h in range(nsplit):
                    pp = P // nsplit
                    xsrc = bass.AP(
                        tensor=X8_d.tensor,
                        offset=a * N * DOUT + h * pp * IC * DOUT,
                        ap=[[IC * DOUT, pp], [1, IC * DOUT]],
                    )
                    nc.gpsimd.dma_start(x_t[h * pp : (h + 1) * pp, :, :], xsrc)
                # SA[a] as [d, j]
                sa_t = sapool.tile([P, N], bf16, tag="SA")
                sasrc = bass.AP(
                    tensor=SA_d.tensor,
                    offset=a * DOUT * N,
                    ap=[[N, DOUT], [1, N]],
                )
                nc.sync.dma_start(sa_t[:], sasrc)
                # psum[d, j] = sum_i X8[a][i,d] * R8[i,j]  (fp8 DoubleRow)
                py = ypool.tile([P, N], f32, tag="py")
                for c4 in range(IC // 2):
                    for jb in range(2):
                        nc.tensor.matmul(
                            py[:, jb * 512 : (jb + 1) * 512],
                            lhsT=x_t[:, 2 * c4 : 2 * c4 + 2, :],
                            rhs=R8_sbs[c4][:, :, jb * 512 : (jb + 1) * 512],
                            start=(c4 == 0),
                            stop=(c4 == IC // 2 - 1),
                            perf_mode=DR,
                        )
                # z2 = psum .* SA (DVE); outcol[d] = sum_j z2 (ACT accum)
                z2_t = zpool.tile([P, N], bf16, tag="Z2")
                nc.vector.tensor_mul(z2_t[:], py[:], sa_t[:])
                tr_t = zpool.tile([P, N], bf16, tag="trash")
                nc.scalar.activation(
                    tr_t[:], z2_t[:], Copy, accum_out=outcols_sb[:, a : a + 1]
                )
                # pipelined finish: as soon as a half of outcols is complete,
                # transpose it (PE quadrant-addressed so partitions align),
                # apply corrections, and store — overlaps the tail with the
                # remaining rows' bulk work
                if a in (ROWS // 2 - 1, ROWS - 1):
                    half = a // (ROWS // 2)
                    HR = ROWS // 2
                    hs = slice(half * HR, half * HR + HR)
                    hd = slice(half * DOUT, (half + 1) * DOUT)
                    pt = spool.tile([P, 512], f32, tag="ps", name=f"ptr{half}")
                    nc.tensor.transpose(pt[:HR, :P], outcols_sb[:, hs], id_sb[:])
                    nc.vector.tensor_add(out_sb[:, hd], pt[:HR, :DOUT], corr_sb[:, hd])
                    nc.vector.tensor_scalar_mul(
                        out_sb[:, hd], out_sb[:, hd], inv_sb[:, half : half + 1]
                    )
                    nc.sync.dma_start(out_d[hs, :], out_sb[:, hd])

    nc.compile()
    return nc


def _prep_inputs(inputs):
    """Host-side sharding + O(N^2 D) prep. Returns per-core input maps."""
    import ml_dtypes

    bf = ml_dtypes.bfloat16
    f8 = ml_dtypes.float8_e4m3
    nf = np.asarray(inputs["node_features"], dtype=np.float32)
    adj = np.asarray(inputs["adjacency_matrix"], dtype=np.float32)
    mf = np.asarray(inputs["mask_father"], dtype=np.float32)[:, 0, :]
    ncnt = np.asarray(inputs["neighbor_count"], dtype=np.float32)
    mh = np.asarray(inputs["mask_hadamard"], dtype=np.float32)[:, 0, :]
    w = np.asarray(inputs["weight"], dtype=np.float32)

    IC = N // P
    wf = nf @ w  # [N, D]
    wfT = np.ascontiguousarray(wf.T)  # [D, N]
    c = mh.mean(axis=0, dtype=np.float64).astype(np.float32)  # [N]
    r = adj.mean(axis=1, dtype=np.float64).astype(np.float32)  # [N]
    R = mh - c[None, :]
    R8 = np.ascontiguousarray(R).astype(f8)
    G2 = (wfT @ R) * wfT  # [D, N]
    G2T = np.ascontiguousarray(G2.T)  # [N, D]

    in_maps = []
    for core in range(NCORES):
        rows = slice(core * ROWS, (core + 1) * ROWS)
        adj_c = adj[rows]
        mf_c = mf[rows]
        r_c = r[rows]
        X8f = (adj_c - r_c[:, None])[:, :, None] * wf[None, :, :]  # [A, N, D]
        # device layout [a, p, c*D+d] with i = c*128+p
        X8 = np.ascontiguousarray(
            X8f.reshape(ROWS, IC, P, DOUT).transpose(0, 2, 1, 3).reshape(
                ROWS, P, IC * DOUT
            )
        ).astype(f8)
        SA = (wfT[None, :, :] * mf_c[:, None, :]).astype(bf)
        F = adj_c @ wf
        H = (mf_c * c[None, :]) @ wf
        K = mf_c @ G2T
        corr = F * H + r_c[:, None] * K
        inv2 = (1.0 / (ncnt[rows] ** 2)).astype(np.float32)  # [ROWS, 1]
        in_maps.append(
            {
                "R8": R8,
                "X8": X8,
                "SA": SA,
                # half-split layouts: [p, h*D+d] = corr[h*64+p, d]
                "corr": np.ascontiguousarray(
                    corr.reshape(2, ROWS // 2, DOUT)
                    .transpose(1, 0, 2)
                    .reshape(ROWS // 2, 2 * DOUT)
                ),
                "inv2": np.ascontiguousarray(
                    inv2.reshape(2, ROWS // 2).T
                ),
            }
        )
    return in_maps


def _run(inputs, trace=False):
    from concourse import bass_utils

    if "k" not in _CACHE:
        _CACHE["k"] = _build()
    nc = _CACHE["k"]
    in_maps = _prep_inputs(inputs)
    res = bass_utils.run_bass_kernel_spmd(
        nc, in_maps, core_ids=list(range(NCORES)), trace=trace
    )
    out = np.concatenate([r["out"] for r in res.results], axis=0)
    return out, res


def kernel(**inputs):
    out, _ = _run(inputs, trace=False)
    return out


# revision 35
# speedup vs baseline: 1.0045x; 1.0045x over previous
"""Trainium2 Bass kernel for masked graph-convolution interaction.

Math (reference):
    wf = node_features @ weight                              # [N, D]
    T[i,d,j] = wf[i,d] * wf[j,d] * mh[i,j]
    S[a,d,j] = sum_i adj[a,i] * T[i,d,j]
    out[a,d] = sum_j S[a,d,j] * mf[a,j] / ncnt[a]^2

Centered-fp8 formulation. With c[j] = mean_i mh[i,j], r[a] = mean_i adj[a,i],
R = mh - c, A' = adj - r:

    out*nc^2 = BULK + F.*H + r.*K            where
    BULK[a,d] = sum_j ( sum_i A'[a,i]wf[i,d] * R[i,j] ) * wf[j,d] * mf[a,j]
    F = adj@wf ; H = (mf.*c)@wf ; G = wf^T@R ; K = mf@(G.*wf^T)^T

The O(N^3 D) BULK runs on-device as fp8 DoubleRow matmuls (2x bf16 rate);
centering halves both operand magnitudes, quartering fp8 quantization error
(~6e-3 rel vs 2.8e-2 naive fp8). Everything O(N^2 D) — wf, the per-row
stationary X8[a] = (adj[a,:]-r)*wf, the post-multiplier SA[a] = wf^T.*mf[a,:],
and the exact correction corr = F.*H + r.*K — is host-prepped (the sharding
hint replicates host-computed wf), so the device pipeline per output row is:

    PE : psum[d,j] = sum_i X8[a][i,d] * R8[i,j]   (8 DoubleRow matmuls)
    DVE: z2 = psum .* SA[a]                       (one [128,1024] multiply)
    ACT: outcol[d] = sum_j z2[d,j]                (free-axis accumulator)

Final: PE transpose of outcols, add corr, scale by 1/nc^2 (host-sent).

Sharding: row-split of a across 8 cores (128 rows each); R8 replicated.
"""

import numpy as np

N = 1024
DIN = 256
DOUT = 128
NCORES = 8
ROWS = N // NCORES  # 128 output rows per core
P = 128

_DTYPE = "fp8-centered"  # informational; test.py prints it

_CACHE = {}


def _build():
    """Build + compile the Bass module (shared across all 8 cores, SPMD)."""
    import concourse.bass as bass
    import concourse.tile as tile
    from concourse import bacc, mybir
    from concourse._compat import axon_active
    from concourse.masks import make_identity

    f32 = mybir.dt.float32
    bf16 = mybir.dt.bfloat16
    f8 = mybir.dt.float8e4
    Copy = mybir.ActivationFunctionType.Copy
    DR = mybir.MatmulPerfMode.DoubleRow

    nc = bacc.Bacc(
        "TRN2",
        target_bir_lowering=False,
        debug=not axon_active(),
        num_devices=NCORES,
    )

    IC = N // P  # 8 i-chunks of 128

    R8_d = nc.dram_tensor("R8", [N, N], f8, kind="ExternalInput").ap()
    # X8 host layout: [a, p, c, d] with i = c*128+p, so each partition's
    # 1KB (IC*DOUT fp8) is one contiguous DMA run
    X8_d = nc.dram_tensor("X8", [ROWS, P, IC * DOUT], f8, kind="ExternalInput").ap()
    SA_d = nc.dram_tensor("SA", [ROWS, DOUT, N], bf16, kind="ExternalInput").ap()
    corr_d = nc.dram_tensor("corr", [ROWS, DOUT], f32, kind="ExternalInput").ap()
    inv2_d = nc.dram_tensor("inv2", [ROWS, 1], f32, kind="ExternalInput").ap()
    out_d = nc.dram_tensor("out", [ROWS, DOUT], f32, kind="ExternalOutput").ap()

    with tile.TileContext(nc) as tc:
        with (
            tc.tile_pool(name="const", bufs=1) as cpool,
            tc.tile_pool(name="x", bufs=6) as xpool,
            tc.tile_pool(name="sa", bufs=6) as sapool,
            tc.tile_pool(name="z", bufs=4) as zpool,
            tc.tile_pool(name="ps", bufs=2, space="PSUM") as spool,
            tc.tile_pool(name="py", bufs=2, space="PSUM") as ypool,
        ):
            # ---- resident tiles + input DMA ----
            # R8 as 4 tiles of 2 i-chunks each: the first DoubleRow matmul
            # only waits on tile 0 (256KB), not the whole 1MB
            R8_sbs = [
                cpool.tile([P, 2, N], f8, tag=f"R8_{c4}", name=f"R8_{c4}")
                for c4 in range(IC // 2)
            ]
            for c4 in range(IC // 2):
                for s in range(2):
                    c = 2 * c4 + s
                    nc.sync.dma_start(
                        R8_sbs[c4][:, s, :], R8_d[c * P : (c + 1) * P, :]
                    )
            corr_sb = cpool.tile([P, DOUT], f32, tag="corr")
            inv_sb = cpool.tile([P, 1], f32, tag="inv")
            nc.sync.dma_start(corr_sb[:], corr_d[:])
            nc.sync.dma_start(inv_sb[:], inv2_d[:])
            id_sb = cpool.tile([P, P], f32, tag="ident")

            # ---- main loop over the 128 output rows ----
            outcols_sb = cpool.tile([P, ROWS], f32, tag="outcols")
            for a in range(ROWS):
                # X8[a] as [p, (c, d)] — contiguous 1KB per partition
                x_t = xpool.tile([P, IC, DOUT], f8, tag="X")
                xsrc = bass.AP(
                    tensor=X8_d.tensor,
                    offset=a * N * DOUT,
                    ap=[[IC * DOUT, P], [1, IC * DOUT]],
                )
                nc.gpsimd.dma_start(x_t[:], xsrc)
                # SA[a] as [d, j]
                sa_t = sapool.tile([P, N], bf16, tag="SA")
                sasrc = bass.AP(
                    tensor=SA_d.tensor,
                    offset=a * DOUT * N,
                    ap=[[N, DOUT], [1, N]],
                )
                nc.sync.dma_start(sa_t[:], sasrc)
                # psum[d, j] = sum_i X8[a][i,d] * R8[i,j]  (fp8 DoubleRow)
                py = ypool.tile([P, N], f32, tag="py")
                for c4 in range(IC // 2):
                    for jb in range(2):
                        nc.tensor.matmul(
                            py[:, jb * 512 : (jb + 1) * 512],
                            lhsT=x_t[:, 2 * c4 : 2 * c4 + 2, :],
                            rhs=R8_sbs[c4][:, :, jb * 512 : (jb + 1) * 512],
                            start=(c4 == 0),
                            stop=(c4 == IC // 2 - 1),
                            perf_mode=DR,
                        )
                # z2 = psum .* SA (DVE); outcol[d] = sum_j z2 (ACT accum)
                z2_t = zpool.tile([P, N], bf16, tag="Z2")
                nc.vector.tensor_mul(z2_t[:], py[:], sa_t[:])
                tr_t = zpool.tile([P, N], bf16, tag="trash")
                nc.scalar.activation(
                    tr_t[:], z2_t[:], Copy, accum_out=outcols_sb[:, a : a + 1]
                )

            # ---- finish: transpose outcols -> [a, d], corrections, store ----
            # (identity built here so its gpsimd ops don't delay the first
            # x_t DMA issued from the gpsimd queue; it still completes long
            # before the transpose needs it)
            make_identity(nc, id_sb[:])
            pt = spool.tile([P, 512], f32, tag="ps", name="ptr")
            nc.tensor.transpose(pt[:, :P], outcols_sb[:], id_sb[:])
            out_sb = cpool.tile([ROWS, DOUT], f32, tag="out_sb")
            nc.vector.tensor_add(out_sb[:], pt[:, :DOUT], corr_sb[:])
            nc.vector.tensor_scalar_mul(out_sb[:], out_sb[:], inv_sb[:])
            nc.sync.dma_start(out_d[:], out_sb[:])

    nc.compile()
    return nc


def _prep_inputs(inputs):
    """Host-side sharding + O(N^2 D) prep. Returns per-core input maps."""
    import ml_dtypes

    bf = ml_dtypes.bfloat16
    f8 = ml_dtypes.float8_e4m3
    nf = np.asarray(inputs["node_features"], dtype=np.float32)
    adj = np.asarray(inputs["adjacency_matrix"], dtype=np.float32)
    mf = np.asarray(inputs["mask_father"], dtype=np.float32)[:, 0, :]
    ncnt = np.asarray(inputs["neighbor_count"], dtype=np.float32)
    mh = np.asarray(inputs["mask_hadamard"], dtype=np.float32)[:, 0, :]
    w = np.asarray(inputs["weight"], dtype=np.float32)

    IC = N // P
    wf = nf @ w  # [N, D]
    wfT = np.ascontiguousarray(wf.T)  # [D, N]
    c = mh.mean(axis=0, dtype=np.float64).astype(np.float32)  # [N]
    r = adj.mean(axis=1, dtype=np.float64).astype(np.float32)  # [N]
    R = mh - c[None, :]
    R8 = np.ascontiguousarray(R).astype(f8)
    G2 = (wfT @ R) * wfT  # [D, N]
    G2T = np.ascontiguousarray(G2.T)  # [N, D]

    in_maps = []
    for core in range(NCORES):
        rows = slice(core * ROWS, (core + 1) * ROWS)
        adj_c = adj[rows]
        mf_c = mf[rows]
        r_c = r[rows]
        X8f = (adj_c - r_c[:, None])[:, :, None] * wf[None, :, :]  # [A, N, D]
        # device layout [a, p, c*D+d] with i = c*128+p
        X8 = np.ascontiguousarray(
            X8f.reshape(ROWS, IC, P, DOUT).transpose(0, 2, 1, 3).reshape(
                ROWS, P, IC * DOUT
            )
        ).astype(f8)
        SA = (wfT[None, :, :] * mf_c[:, None, :]).astype(bf)
        F = adj_c @ wf
        H = (mf_c * c[None, :]) @ wf
        K = mf_c @ G2T
        corr = F * H + r_c[:, None] * K
        in_maps.append(
            {
                "R8": R8,
                "X8": X8,
                "SA": SA,
                "corr": np.ascontiguousarray(corr),
                "inv2": np.ascontiguousarray(
                    (1.0 / (ncnt[rows] ** 2)).astype(np.float32)
                ),
            }
        )
    return in_maps


def _run(inputs, trace=False):
    from concourse import bass_utils

    if "k" not in _CACHE:
        _CACHE["k"] = _build()
    nc = _CACHE["k"]
    in_maps = _prep_inputs(inputs)
    res = bass_utils.run_bass_kernel_spmd(
        nc, in_maps, core_ids=list(range(NCORES)), trace=trace
    )
    out = np.concatenate([r["out"] for r in res.results], axis=0)
    return out, res


def kernel(**inputs):
    out, _ = _run(inputs, trace=False)
    return out


# revision 38
# speedup vs baseline: 1.0084x; 1.0039x over previous
"""Trainium2 Bass kernel for masked graph-convolution interaction.

Math (reference):
    wf = node_features @ weight                              # [N, D]
    T[i,d,j] = wf[i,d] * wf[j,d] * mh[i,j]
    S[a,d,j] = sum_i adj[a,i] * T[i,d,j]
    out[a,d] = sum_j S[a,d,j] * mf[a,j] / ncnt[a]^2

Centered-fp8 formulation. With c[j] = mean_i mh[i,j], r[a] = mean_i adj[a,i],
R = mh - c, A' = adj - r:

    out*nc^2 = BULK + F.*H + r.*K            where
    BULK[a,d] = sum_j ( sum_i A'[a,i]wf[i,d] * R[i,j] ) * wf[j,d] * mf[a,j]
    F = adj@wf ; H = (mf.*c)@wf ; G = wf^T@R ; K = mf@(G.*wf^T)^T

The O(N^3 D) BULK runs on-device as fp8 DoubleRow matmuls (2x bf16 rate);
centering halves both operand magnitudes, quartering fp8 quantization error
(~6e-3 rel vs 2.8e-2 naive fp8). Everything O(N^2 D) — wf, the per-row
stationary X8[a] = (adj[a,:]-r)*wf, the post-multiplier SA[a] = wf^T.*mf[a,:],
and the exact correction corr = F.*H + r.*K — is host-prepped (the sharding
hint replicates host-computed wf), so the device pipeline per output row is:

    PE : psum[d,j] = sum_i X8[a][i,d] * R8[i,j]   (8 DoubleRow matmuls)
    DVE: z2 = psum .* SA[a]                       (one [128,1024] multiply)
    ACT: outcol[d] = sum_j z2[d,j]                (free-axis accumulator)

Final: PE transpose of outcols, add corr, scale by 1/nc^2 (host-sent).

Sharding: row-split of a across 8 cores (128 rows each); R8 replicated.
"""

import numpy as np

N = 1024
DIN = 256
DOUT = 128
NCORES = 8
ROWS = N // NCORES  # 128 output rows per core
P = 128

_DTYPE = "fp8-centered"  # informational; test.py prints it

_CACHE = {}


def _build():
    """Build + compile the Bass module (shared across all 8 cores, SPMD)."""
    import concourse.bass as bass
    import concourse.tile as tile
    from concourse import bacc, mybir
    from concourse._compat import axon_active
    from concourse.masks import make_identity

    f32 = mybir.dt.float32
    bf16 = mybir.dt.bfloat16
    f8 = mybir.dt.float8e4
    Copy = mybir.ActivationFunctionType.Copy
    DR = mybir.MatmulPerfMode.DoubleRow

    nc = bacc.Bacc(
        "TRN2",
        target_bir_lowering=False,
        debug=not axon_active(),
        num_devices=NCORES,
    )

    IC = N // P  # 8 i-chunks of 128

    R8_d = nc.dram_tensor("R8", [N, N], f8, kind="ExternalInput").ap()
    # X8 host layout: [a, p, c, d] with i = c*128+p, so each partition's
    # 1KB (IC*DOUT fp8) is one contiguous DMA run
    X8_d = nc.dram_tensor("X8", [ROWS, P, IC * DOUT], f8, kind="ExternalInput").ap()
    SA_d = nc.dram_tensor("SA", [ROWS, DOUT, N], bf16, kind="ExternalInput").ap()
    corr_d = nc.dram_tensor("corr", [ROWS, DOUT], f32, kind="ExternalInput").ap()
    inv2_d = nc.dram_tensor("inv2", [ROWS, 1], f32, kind="ExternalInput").ap()
    out_d = nc.dram_tensor("out", [ROWS, DOUT], f32, kind="ExternalOutput").ap()

    with tile.TileContext(nc) as tc:
        with (
            tc.tile_pool(name="const", bufs=1) as cpool,
            tc.tile_pool(name="x", bufs=6) as xpool,
            tc.tile_pool(name="sa", bufs=6) as sapool,
            tc.tile_pool(name="z", bufs=4) as zpool,
            tc.tile_pool(name="ps", bufs=2, space="PSUM") as spool,
            tc.tile_pool(name="py", bufs=2, space="PSUM") as ypool,
        ):
            # ---- resident tiles + input DMA ----
            # R8 as 4 tiles of 2 i-chunks each: the first DoubleRow matmul
            # only waits on tile 0 (256KB), not the whole 1MB
            R8_sbs = [
                cpool.tile([P, 2, N], f8, tag=f"R8_{c4}", name=f"R8_{c4}")
                for c4 in range(IC // 2)
            ]
            for c4 in range(IC // 2):
                for s in range(2):
                    c = 2 * c4 + s
                    nc.sync.dma_start(
                        R8_sbs[c4][:, s, :], R8_d[c * P : (c + 1) * P, :]
                    )
            corr_sb = cpool.tile([P, DOUT], f32, tag="corr")
            inv_sb = cpool.tile([P, 1], f32, tag="inv")
            nc.sync.dma_start(corr_sb[:], corr_d[:])
            nc.sync.dma_start(inv_sb[:], inv2_d[:])
            id_sb = cpool.tile([P, P], f32, tag="ident")

            # ---- main loop over the 128 output rows ----
            outcols_sb = cpool.tile([P, ROWS], f32, tag="outcols")
            for a in range(ROWS):
                # X8[a] as [p, (c, d)] — contiguous 1KB per partition
                x_t = xpool.tile([P, IC, DOUT], f8, tag="X")
                xsrc = bass.AP(
                    tensor=X8_d.tensor,
                    offset=a * N * DOUT,
                    ap=[[IC * DOUT, P], [1, IC * DOUT]],
                )
                nc.gpsimd.dma_start(x_t[:], xsrc)
                # SA[a] as [d, j]
                sa_t = sapool.tile([P, N], bf16, tag="SA")
                sasrc = bass.AP(
                    tensor=SA_d.tensor,
                    offset=a * DOUT * N,
                    ap=[[N, DOUT], [1, N]],
                )
                nc.sync.dma_start(sa_t[:], sasrc)
                # psum[d, j] = sum_i X8[a][i,d] * R8[i,j]  (fp8 DoubleRow)
                py = ypool.tile([P, N], f32, tag="py")
                for c4 in range(IC // 2):
                    for jb in range(2):
                        nc.tensor.matmul(
                            py[:, jb * 512 : (jb + 1) * 512],
                            lhsT=x_t[:, 2 * c4 : 2 * c4 + 2, :],
                            rhs=R8_sbs[c4][:, :, jb * 512 : (jb + 1) * 512],
                            start=(c4 == 0),
                            stop=(c4 == IC // 2 - 1),
                            perf_mode=DR,
                        )
                # z2 = psum .* SA (DVE); outcol[d] = sum_j z2 (ACT accum)
                z2_t = zpool.tile([P, N], bf16, tag="Z2")
                nc.vector.tensor_mul(z2_t[:], py[:], sa_t[:])
                tr_t = zpool.tile([P, N], bf16, tag="trash")
                nc.scalar.activation(
                    tr_t[:], z2_t[:], Copy, accum_out=outcols_sb[:, a : a + 1]
                )

            # ---- finish: transpose outcols -> [a, d], corrections, store ----
            # (identity built here so its gpsimd ops don't delay the first
            # x_t DMA issued from the gpsimd queue; it still completes long
            # before the transpose needs it)
            make_identity(nc, id_sb[:])
            pt = spool.tile([P, 512], f32, tag="ps", name="ptr")
            nc.tensor.transpose(pt[:, :P], outcols_sb[:], id_sb[:])
            out_sb = cpool.tile([ROWS, DOUT], f32, tag="out_sb")
            nc.vector.tensor_add(out_sb[:], pt[:, :DOUT], corr_sb[:])
            nc.vector.tensor_scalar_mul(out_sb[:], out_sb[:], inv_sb[:])
            nc.sync.dma_start(out_d[:], out_sb[:])

    nc.compile()
    return nc


def _prep_inputs(inputs):
    """Host-side sharding + O(N^2 D) prep. Returns per-core input maps."""
    import ml_dtypes

    bf = ml_dtypes.bfloat16
    f8 = ml_dtypes.float8_e4m3
    nf = np.asarray(inputs["node_features"], dtype=np.float32)
    adj = np.asarray(inputs["adjacency_matrix"], dtype=np.float32)
    mf = np.asarray(inputs["mask_father"], dtype=np.float32)[:, 0, :]
    ncnt = np.asarray(inputs["neighbor_count"], dtype=np.float32)
    mh = np.asarray(inputs["mask_hadamard"], dtype=np.float32)[:, 0, :]
    w = np.asarray(inputs["weight"], dtype=np.float32)

    IC = N // P
    wf = nf @ w  # [N, D]
    wfT = np.ascontiguousarray(wf.T)  # [D, N]
    c = mh.mean(axis=0, dtype=np.float64).astype(np.float32)  # [N]
    r = adj.mean(axis=1, dtype=np.float64).astype(np.float32)  # [N]
    R = mh - c[None, :]
    R8 = np.ascontiguousarray(R).astype(f8)
    G2 = (wfT @ R) * wfT  # [D, N]
    G2T = np.ascontiguousarray(G2.T)  # [N, D]

    in_maps = []
    for core in range(NCORES):
        rows = slice(core * ROWS, (core + 1) * ROWS)
        adj_c = adj[rows]
        mf_c = mf[rows]
        r_c = r[rows]
        X8f = (adj_c - r_c[:, None])[:, :, None] * wf[None, :, :]  # [A, N, D]
        # device layout [a, p, c*D+d] with i = c*128+p
        X8 = np.ascontiguousarray(
            X8f.reshape(ROWS, IC, P, DOUT).transpose(0, 2, 1, 3).reshape(
                ROWS, P, IC * DOUT
            )
        ).astype(f8)
        SA = (wfT[None, :, :] * mf_c[:, None, :]).astype(bf)
        F = adj_c @ wf
        H = (mf_c * c[None, :]) @ wf
        K = mf_c @ G2T
        corr = F * H + r_c[:, None] * K
        in_maps.append(
            {
                "R8": R8,
                "X8": X8,
                "SA": SA,
                "corr": np.ascontiguousarray(corr),
                "inv2": np.ascontiguousarray(
                    (1.0 / (ncnt[rows] ** 2)).astype(np.float32)
                ),
            }
        )
    return in_maps


def _run(inputs, trace=False):
    from concourse import bass_utils

    if "k" not in _CACHE:
        _CACHE["k"] = _build()
    nc = _CACHE["k"]
    in_maps = _prep_inputs(inputs)
    res = bass_utils.run_bass_kernel_spmd(
        nc, in_maps, core_ids=list(range(NCORES)), trace=trace
    )
    out = np.concatenate([r["out"] for r in res.results], axis=0)
    return out, res


def kernel(**inputs):
    out, _ = _run(inputs, trace=False)
    return out
